# revision 4
# baseline (speedup 1.0000x reference)
"""Self-contained Trainium2 (Bass/Tile) kernel: single-head causal attention.

Problem: embeddings [4,4096,1024] f32; Wq/Wk/Wv [1024,1024] f32 (torch Linear
layout [out,in]).  out = softmax(causal(QK^T)/sqrt(D)) @ V, computed per batch.

Distribution (v3): 8 NeuronCores, one SPMD program, context-parallel split.
Core c handles batch c//2; the two cores of a batch pair split the KEY axis by
interleaved 128-row k-tiles (even core: true tiles 0,2,4,...; odd: 1,3,5,...).
Each core processes ALL 4096 query rows against its 2048 k-rows and emits
unnormalized partial attention (sum of exp-weights times V) plus the partial
softmax denominators; the host unshard step adds the pair's partials and
divides.  This is the standard sequence/context-parallel attention combine.

Per-core work: slot j = query chunk j (256 rows) needs exactly j+1 of this
core's k-tiles (perfect causal balance; only each slot's last tile is
diagonal-masked, via a per-core mask table input).  V is projected only for
this core's k-rows (no duplicate work in the pair) and stays resident in
SBUF.  scores = emb_q M emb_k^T with M = Wq^T Wk folded on the K side:
KP = M @ embk^T, so neither Q nor K is ever materialized.

Host-side staging: transpose + bf16-cast, k-tile gather for embk, mask table,
and the final pair combine (add partials, divide by summed denominator).
All matmuls (M, V, KP, scores, AV) and the exp run on device in bf16 with f32
accumulation.
"""

import math
import os
import sys
import types

import numpy as np
import ml_dtypes

B, S, D = 4, 4096, 1024
NCORES = 8
NSLOT = 16
CHUNK = 256          # q rows per slot
KHALF = S // 2       # k rows owned per core
INV_SQRT_D = 1.0 / math.sqrt(D)
BF16 = ml_dtypes.bfloat16


# ---------------------------------------------------------------------------
# Environment patches (compiler workarounds + profiling hook shim)
# ---------------------------------------------------------------------------

def _install_patches():
    import json as _json
    import concourse.bass as bass

    if not getattr(bass.Bass, "_mw_patched", False):
        _orig_to_json = bass.Bass.to_json_bytes

        def to_json_bytes(self):
            # This walrus build rejects any instruction carrying more than one
            # sync wait ("Too many sync wait commands").  Split extra waits
            # onto single-wait NoOps inserted just before the instruction (the
            # engine executes them in order, so semantics are unchanged).
            raw = _orig_to_json(self)
            m = _json.loads(raw)
            ctr = 0
            changed = False
            for fn in m.get("functions", []):
                for bb in fn.get("blocks", []):
                    out = []
                    for inst in bb.get("instructions", []):
                        si = inst.get("sync_info")
                        if si:
                            waits = si.get("on_wait") or []
                            if len(waits) > 1:
                                changed = True
                                for w in waits[:-1]:
                                    ctr += 1
                                    out.append({
                                        "debug": inst.get("debug", 0),
                                        "engine": inst["engine"],
                                        "ins": [],
                                        "outs": [],
                                        "name": f"I-mw{ctr}",
                                        "opcode": "NoOp",
                                        "text_hint": "mwsplit",
                                        "sync_info": {"on_wait": [w],
                                                      "on_update": []},
                                    })
                                si["on_wait"] = [waits[-1]]
                        out.append(inst)
                    bb["instructions"] = out
            if not changed:
                return raw
            return _json.dumps(m).encode()

        bass.Bass.to_json_bytes = to_json_bytes
        bass.Bass._mw_patched = True

    # Don't upload NEFF/trace artifacts anywhere; keep them local.
    import concourse.bass_utils as bu
    bu.upload_artifacts = lambda tmpdir: tmpdir


def _install_ntff_hook() -> bool:
    """Register the axon NTFF profiling hook (missing module in this image)."""
    try:
        import antenv.axon_hooks  # noqa: F401
        return True
    except ImportError:
        pass
    try:
        mod = types.ModuleType("antenv.axon_hooks")
        state = {"hook": None}
        mod.set_axon_ntff_profile_hook = lambda h: state.__setitem__("hook", h)
        mod.get_axon_ntff_profile_hook = lambda: state["hook"]
        sys.modules["antenv.axon_hooks"] = mod
        import antenv
        antenv.axon_hooks = mod
        from trn_agent_boot.trn_boot import _ntff_profile_via_ctypes
        mod.set_axon_ntff_profile_hook(
            _ntff_profile_via_ctypes("/opt/axon/libaxon_pjrt.so"))
        return True
    except Exception:
        return False


# ---------------------------------------------------------------------------
# Graph
# ---------------------------------------------------------------------------

def _build_graph():
    import concourse.bass as bass
    import concourse.mybir as mybir
    import concourse.tile as tile

    f32 = mybir.dt.float32
    bf16 = mybir.dt.bfloat16
    Exp = mybir.ActivationFunctionType.Exp

    nc = bass.Bass("TRN2", debug=False, num_devices=NCORES)

    embT_in = nc.dram_tensor("embT", [D, S], bf16, kind="ExternalInput")
    embkT_in = nc.dram_tensor("embkT", [D, KHALF], bf16, kind="ExternalInput")
    wq_in = nc.dram_tensor("wqn", [D, D], bf16, kind="ExternalInput")
    wk_in = nc.dram_tensor("wkn", [D, D], bf16, kind="ExternalInput")
    wvT_in = nc.dram_tensor("wvT", [D, D], bf16, kind="ExternalInput")
    masks_in = nc.dram_tensor("masks", [128, NSLOT * CHUNK], bf16,
                              kind="ExternalInput")
    oav_d = nc.dram_tensor("oav", [S, D], f32, kind="ExternalOutput")
    ol_d = nc.dram_tensor("ol", [NSLOT, 128, 2], f32, kind="ExternalOutput")

    with tile.TileContext(nc) as tc:
        with (
            tc.tile_pool(name="wsb", bufs=1) as wsb,          # weights resident
            tc.tile_pool(name="eksb", bufs=1) as eksb,        # embk^T resident
            tc.tile_pool(name="vsb", bufs=1) as vsb,          # V resident
            tc.tile_pool(name="eqs", bufs=16) as eqs,         # embT q-side stream
            tc.tile_pool(name="mks", bufs=1) as mks,          # masks resident
            tc.tile_pool(name="wts", bufs=6) as wts,          # exp weights
            tc.tile_pool(name="outs", bufs=3) as outs,        # av out stage
            tc.tile_pool(name="smalls", bufs=4) as smalls,
            tc.tile_pool(name="pmm", bufs=2, space="PSUM") as pmm,
            tc.tile_pool(name="ps", bufs=2, space="PSUM") as ps_pool,
            tc.tile_pool(name="pl", bufs=1, space="PSUM") as pl_pool,
        ):
            # constants
            ones = smalls.tile([128, 1], bf16, name="ones", tag="ones")
            nc.gpsimd.memset(ones[:], 1.0)

            # resident weight tiles; DMA order = need order (M first)
            wk_n, wq_n, wv_t = [], [], []
            for dc in range(8):
                t = wsb.tile([128, D], bf16, name=f"wk{dc}", tag=f"wk{dc}")
                nc.sync.dma_start(t[:], wk_in[dc * 128:(dc + 1) * 128, :])
                wk_n.append(t)
                t = wsb.tile([128, D], bf16, name=f"wq{dc}", tag=f"wq{dc}")
                nc.sync.dma_start(t[:], wq_in[dc * 128:(dc + 1) * 128, :])
                wq_n.append(t)
            embk_sb = []
            for dc in range(8):
                t = eksb.tile([128, KHALF], bf16, name=f"ek{dc}",
                              tag=f"ek{dc}")
                nc.sync.dma_start(t[:], embkT_in[dc * 128:(dc + 1) * 128, :])
                embk_sb.append(t)
            for dc in range(8):
                t = wsb.tile([128, D], bf16, name=f"wv{dc}", tag=f"wv{dc}")
                nc.sync.dma_start(t[:], wvT_in[dc * 128:(dc + 1) * 128, :])
                wv_t.append(t)

            # ---------------- MT = Wk^T @ Wq  [d', d] ----------------
            # scores = emb_q M emb_k^T with M[d,d'] = sum_e Wq[e,d] Wk[e,d'];
            # we materialize M^T (tiles [128d', 1024d]) as the lhsT source for
            # the K-side fold below.
            mt_sb = []
            for ac in range(8):
                psum = pmm.tile([128, 1024], f32, name=f"pm{ac}", tag="mm")
                for bb in range(2):
                    for ec in range(8):
                        nc.tensor.matmul(
                            psum[:, bb * 512:(bb + 1) * 512],
                            wk_n[ec][:, ac * 128:(ac + 1) * 128],
                            wq_n[ec][:, bb * 512:(bb + 1) * 512],
                            start=(ec == 0), stop=(ec == 7))
                t = wsb.tile([128, D], bf16, name=f"mt{ac}", tag=f"mt{ac}")
                nc.scalar.copy(t[:], psum[:])
                mt_sb.append(t)

            # ---------------- V projection (my k rows), SBUF resident -------
            v_sb = []
            for sr in range(16):
                psum = pmm.tile([128, 1024], f32, name=f"pv{sr}", tag="mm")
                col = sr * 128
                for eb in range(2):
                    for dc in range(8):
                        nc.tensor.matmul(
                            psum[:, eb * 512:(eb + 1) * 512],
                            embk_sb[dc][:, col:col + 128],
                            wv_t[dc][:, eb * 512:(eb + 1) * 512],
                            start=(dc == 0), stop=(dc == 7))
                t = vsb.tile([128, 1024], bf16, name=f"v{sr}", tag=f"v{sr}")
                nc.scalar.copy(t[:], psum[:])
                v_sb.append(t)

            # ---------------- KP = M @ embk^T  [d, k] (scores lhsT) ---------
            # KP[d,k] = sum_d' MT[d',d] embk^T[d',k].  16 half-tiles
            # [128d, 1024k]; kh=0 (k tiles 0..7) first so early slots can
            # start while kh=1 computes.  Aliases the dead wk and wv buffers.
            kp_sb = [None] * 16
            for kh in range(2):
                for dc in range(8):
                    psum = pmm.tile([128, 1024], f32, name=f"pk{kh}_{dc}",
                                    tag="mm")
                    for kb in range(2):
                        koff = kh * 1024 + kb * 512
                        for ec in range(8):
                            nc.tensor.matmul(
                                psum[:, kb * 512:(kb + 1) * 512],
                                mt_sb[ec][:, dc * 128:(dc + 1) * 128],
                                embk_sb[ec][:, koff:koff + 512],
                                start=(ec == 0), stop=(ec == 7))
                    alias = f"wk{dc}" if kh == 0 else f"wv{dc}"
                    t = wsb.tile([128, 1024], bf16, name=f"kp{kh}_{dc}",
                                 tag=alias)
                    nc.scalar.copy(t[:], psum[:])
                    kp_sb[dc * 2 + kh] = t

            # ---------------- attention ----------------
            # slot j = query chunk j (rows 256j..256j+255); k tiles 0..j of
            # this core's half; tile kt lives in kp_sb[dc*2 + kt//8] at column
            # block (kt%8)*128 and v_sb[kt].  Only kt==j is diagonal-masked.
            #
            # Software-pipelined one k-tile deep AND one slot deep: each AV
            # group is emitted after the next scores+exp issue, so the tensor
            # engine never waits on the scalar exp — including at slot
            # boundaries, where the previous slot's last AV group (and its
            # output staging) runs after the next slot's first scores.
            mk_all = mks.tile([128, NSLOT * CHUNK], bf16, name="mka",
                              tag="mka")
            nc.sync.dma_start(mk_all[:], masks_in[:, :])

            def emit_av(l_ps, av, j, wt, kt):
                first, last = kt == 0, kt == j
                vt = v_sb[kt]
                for qs in range(2):
                    wslice = wt[:, qs * 128:(qs + 1) * 128]
                    nc.tensor.matmul(l_ps[qs][:], wslice, ones[:],
                                     start=first, stop=last)
                    for eb in range(2):
                        nc.tensor.matmul(
                            av[qs][:, eb * 512:(eb + 1) * 512], wslice,
                            vt[:, eb * 512:(eb + 1) * 512],
                            start=first, stop=last)

            def finish_slot(prev):
                j, l_ps, av, wt, kt = prev
                emit_av(l_ps, av, j, wt, kt)
                # unnormalized partials; the host divides by the pair-summed
                # denominator.  (DMA can't source PSUM, so stage via SBUF.)
                l_sb = smalls.tile([128, 2], f32, name=f"ls{j}", tag="lst")
                for qs in range(2):
                    nc.vector.tensor_copy(l_sb[:, qs:qs + 1], l_ps[qs][:])
                nc.gpsimd.dma_start(ol_d[j, :, :], l_sb[:])
                for qs in range(2):
                    o_sb = outs.tile([128, 1024], f32, name=f"o{j}_{qs}",
                                     tag="outs")
                    nc.vector.tensor_copy(o_sb[:], av[qs][:])
                    row = (j * 2 + qs) * 128
                    nc.gpsimd.dma_start(oav_d[row:row + 128, :], o_sb[:])

            def scores_exp(eq, j, kt):
                s_ps = ps_pool.tile([128, CHUNK], f32, name=f"s{j}_{kt}",
                                    tag="s")
                half = kt // 8
                kcol = (kt % 8) * 128
                for dc in range(8):
                    nc.tensor.matmul(
                        s_ps[:], kp_sb[dc * 2 + half][:, kcol:kcol + 128],
                        eq[dc][:], start=(dc == 0), stop=(dc == 7))
                wt = wts.tile([128, CHUNK], bf16, name=f"w{j}_{kt}",
                              tag="wts")
                nc.scalar.activation(wt[:], s_ps[:], Exp, bias=0.0,
                                     scale=INV_SQRT_D)
                if kt == j:
                    nc.vector.tensor_mul(
                        wt[:], wt[:], mk_all[:, j * CHUNK:(j + 1) * CHUNK])
                return wt

            prev = None
            for j in range(NSLOT):
                eq = []
                for dc in range(8):
                    t = eqs.tile([128, CHUNK], bf16, name=f"eq{j}_{dc}",
                                 tag="eqs")
                    nc.sync.dma_start(
                        t[:], embT_in[dc * 128:(dc + 1) * 128,
                                      j * CHUNK:(j + 1) * CHUNK])
                    eq.append(t)

                wt0 = scores_exp(eq, j, 0)
                if prev is not None:
                    finish_slot(prev)

                l_ps = [pl_pool.tile([128, 1], f32, name=f"l{j}_{qs}",
                                     tag=f"l{qs}") for qs in range(2)]
                av = [pmm.tile([128, 1024], f32, name=f"av{j}_{qs}", tag="mm")
                      for qs in range(2)]

                pend = (wt0, 0)
                for kt in range(1, j + 1):
                    wt = scores_exp(eq, j, kt)
                    emit_av(l_ps, av, j, *pend)
                    pend = (wt, kt)
                prev = (j, l_ps, av) + pend
            finish_slot(prev)

    return nc


_CACHED = {}


def _get_graph():
    if "nc" not in _CACHED:
        _install_patches()
        _CACHED["nc"] = _build_graph()
    return _CACHED["nc"]


# ---------------------------------------------------------------------------
# Host-side staging
# ---------------------------------------------------------------------------

def _masks(parity):
    m = np.zeros((NSLOT, 128, CHUNK), dtype=np.float32)
    for j in range(NSLOT):
        p = np.arange(128)[:, None]
        x = np.arange(CHUNK)[None, :]
        m[j] = ((j * CHUNK + x) >= ((2 * j + parity) * 128 + p))
    # device layout: [128 k-partitions, slot-major free dim]
    return np.ascontiguousarray(
        m.transpose(1, 0, 2).reshape(128, NSLOT * CHUNK)).astype(BF16)


def kernel(embeddings, Wq, Wk, Wv):
    embeddings = np.asarray(embeddings, dtype=np.float32)
    Wq = np.asarray(Wq, dtype=np.float32)
    Wk = np.asarray(Wk, dtype=np.float32)
    Wv = np.asarray(Wv, dtype=np.float32)

    nc = _get_graph()
    from concourse.bass_utils import run_bass_kernel_spmd

    wqn = Wq.astype(BF16)
    wkn = Wk.astype(BF16)
    wvT = np.ascontiguousarray(Wv.T).astype(BF16)
    masks_by_par = [_masks(0), _masks(1)]

    in_maps = []
    for c in range(NCORES):
        b, par = divmod(c, 2)
        emb_b = embeddings[b]
        embT = np.ascontiguousarray(emb_b.T).astype(BF16)
        # my k rows: interleaved 128-row tiles (2t+par for t in 0..15)
        embk = np.concatenate(
            [emb_b[(2 * t + par) * 128:(2 * t + par) * 128 + 128]
             for t in range(16)], axis=0)
        embkT = np.ascontiguousarray(embk.T).astype(BF16)
        in_maps.append({
            "embT": embT,
            "embkT": embkT,
            "wqn": wqn,
            "wkn": wkn,
            "wvT": wvT,
            "masks": masks_by_par[par],
        })

    trace = bool(int(os.environ.get("BASS_KERNEL_TRACE", "0")))
    kwargs = {}
    if trace:
        kwargs["trace"] = _install_ntff_hook()

    res = run_bass_kernel_spmd(nc, in_maps, core_ids=list(range(NCORES)),
                               **kwargs)
    _CACHED["last_result"] = res

    out = np.empty((B, S, D), dtype=np.float32)
    for b in range(B):
        r0, r1 = res.results[2 * b], res.results[2 * b + 1]
        av = r0["oav"] + r1["oav"]                      # [S, D]
        l = (r0["ol"] + r1["ol"])                       # [16, 128, 2]
        lfull = l.transpose(0, 2, 1).reshape(S, 1)      # q = 256j+128qs+p
        out[b] = av / lfull
    return out


# revision 5
# speedup vs baseline: 1.0680x; 1.0680x over previous
"""Self-contained Trainium2 (Bass/Tile) kernel: single-head causal attention.

Problem: embeddings [4,4096,1024] f32; Wq/Wk/Wv [1024,1024] f32 (torch Linear
layout [out,in]).  out = softmax(causal(QK^T)/sqrt(D)) @ V, computed per batch.

Distribution (v3): 8 NeuronCores, one SPMD program, context-parallel split.
Core c handles batch c//2; the two cores of a batch pair split the KEY axis by
interleaved 128-row k-tiles (even core: true tiles 0,2,4,...; odd: 1,3,5,...).
Each core processes ALL 4096 query rows against its 2048 k-rows and emits
unnormalized partial attention (sum of exp-weights times V) plus the partial
softmax denominators; the host unshard step adds the pair's partials and
divides.  This is the standard sequence/context-parallel attention combine.

Per-core work: slot j = query chunk j (256 rows) needs exactly j+1 of this
core's k-tiles (perfect causal balance; only each slot's last tile is
diagonal-masked, via a per-core mask table input).  V is projected only for
this core's k-rows (no duplicate work in the pair) and stays resident in
SBUF.  scores = emb_q M emb_k^T with M = Wq^T Wk folded on the K side:
KP = M @ embk^T, so neither Q nor K is ever materialized.

Host-side staging: transpose + bf16-cast, k-tile gather for embk, mask table,
and the final pair combine (add partials, divide by summed denominator).
All matmuls (M, V, KP, scores, AV) and the exp run on device in bf16 with f32
accumulation.
"""

import math
import os
import sys
import types

import numpy as np
import ml_dtypes

B, S, D = 4, 4096, 1024
NCORES = 8
NSLOT = 16
CHUNK = 256          # q rows per slot
KHALF = S // 2       # k rows owned per core
INV_SQRT_D = 1.0 / math.sqrt(D)
BF16 = ml_dtypes.bfloat16


# ---------------------------------------------------------------------------
# Environment patches (compiler workarounds + profiling hook shim)
# ---------------------------------------------------------------------------

def _install_patches():
    import json as _json
    import concourse.bass as bass

    if not getattr(bass.Bass, "_mw_patched", False):
        _orig_to_json = bass.Bass.to_json_bytes

        def to_json_bytes(self):
            # This walrus build rejects any instruction carrying more than one
            # sync wait ("Too many sync wait commands").  Split extra waits
            # onto single-wait NoOps inserted just before the instruction (the
            # engine executes them in order, so semantics are unchanged).
            raw = _orig_to_json(self)
            m = _json.loads(raw)
            ctr = 0
            changed = False
            for fn in m.get("functions", []):
                for bb in fn.get("blocks", []):
                    out = []
                    for inst in bb.get("instructions", []):
                        si = inst.get("sync_info")
                        if si:
                            waits = si.get("on_wait") or []
                            if len(waits) > 1:
                                changed = True
                                for w in waits[:-1]:
                                    ctr += 1
                                    out.append({
                                        "debug": inst.get("debug", 0),
                                        "engine": inst["engine"],
                                        "ins": [],
                                        "outs": [],
                                        "name": f"I-mw{ctr}",
                                        "opcode": "NoOp",
                                        "text_hint": "mwsplit",
                                        "sync_info": {"on_wait": [w],
                                                      "on_update": []},
                                    })
                                si["on_wait"] = [waits[-1]]
                        out.append(inst)
                    bb["instructions"] = out
            if not changed:
                return raw
            return _json.dumps(m).encode()

        bass.Bass.to_json_bytes = to_json_bytes
        bass.Bass._mw_patched = True

    # Don't upload NEFF/trace artifacts anywhere; keep them local.
    import concourse.bass_utils as bu
    bu.upload_artifacts = lambda tmpdir: tmpdir


def _install_ntff_hook() -> bool:
    """Register the axon NTFF profiling hook (missing module in this image)."""
    try:
        import antenv.axon_hooks  # noqa: F401
        return True
    except ImportError:
        pass
    try:
        mod = types.ModuleType("antenv.axon_hooks")
        state = {"hook": None}
        mod.set_axon_ntff_profile_hook = lambda h: state.__setitem__("hook", h)
        mod.get_axon_ntff_profile_hook = lambda: state["hook"]
        sys.modules["antenv.axon_hooks"] = mod
        import antenv
        antenv.axon_hooks = mod
        from trn_agent_boot.trn_boot import _ntff_profile_via_ctypes
        mod.set_axon_ntff_profile_hook(
            _ntff_profile_via_ctypes("/opt/axon/libaxon_pjrt.so"))
        return True
    except Exception:
        return False


# ---------------------------------------------------------------------------
# Graph
# ---------------------------------------------------------------------------

def _build_graph():
    import concourse.bass as bass
    import concourse.mybir as mybir
    import concourse.tile as tile

    f32 = mybir.dt.float32
    bf16 = mybir.dt.bfloat16
    Exp = mybir.ActivationFunctionType.Exp

    nc = bass.Bass("TRN2", debug=False, num_devices=NCORES)

    embT_in = nc.dram_tensor("embT", [D, S], bf16, kind="ExternalInput")
    embkT_in = nc.dram_tensor("embkT", [D, KHALF], bf16, kind="ExternalInput")
    wq_in = nc.dram_tensor("wqn", [D, D], bf16, kind="ExternalInput")
    wk_in = nc.dram_tensor("wkn", [D, D], bf16, kind="ExternalInput")
    wvT_in = nc.dram_tensor("wvT", [D, D], bf16, kind="ExternalInput")
    masks_in = nc.dram_tensor("masks", [128, NSLOT * CHUNK], bf16,
                              kind="ExternalInput")
    oav_d = nc.dram_tensor("oav", [S, D], f32, kind="ExternalOutput")
    ol_d = nc.dram_tensor("ol", [NSLOT, 128, 2], f32, kind="ExternalOutput")

    with tile.TileContext(nc) as tc:
        with (
            tc.tile_pool(name="wsb", bufs=1) as wsb,          # weights resident
            tc.tile_pool(name="eksb", bufs=1) as eksb,        # embk^T resident
            tc.tile_pool(name="vsb", bufs=1) as vsb,          # V resident
            tc.tile_pool(name="eqs", bufs=16) as eqs,         # embT q-side stream
            tc.tile_pool(name="mks", bufs=1) as mks,          # masks resident
            tc.tile_pool(name="wts", bufs=6) as wts,          # exp weights
            tc.tile_pool(name="outs", bufs=3) as outs,        # av out stage
            tc.tile_pool(name="smalls", bufs=4) as smalls,
            tc.tile_pool(name="pmm", bufs=2, space="PSUM") as pmm,
        ):
            # constants
            ones = smalls.tile([128, 1], bf16, name="ones", tag="ones")
            nc.gpsimd.memset(ones[:], 1.0)

            # resident weight tiles; DMA order = need order (M first)
            wk_n, wq_n, wv_t = [], [], []
            for dc in range(8):
                t = wsb.tile([128, D], bf16, name=f"wk{dc}", tag=f"wk{dc}")
                nc.sync.dma_start(t[:], wk_in[dc * 128:(dc + 1) * 128, :])
                wk_n.append(t)
                t = wsb.tile([128, D], bf16, name=f"wq{dc}", tag=f"wq{dc}")
                nc.sync.dma_start(t[:], wq_in[dc * 128:(dc + 1) * 128, :])
                wq_n.append(t)
            embk_sb = []
            for dc in range(8):
                t = eksb.tile([128, KHALF], bf16, name=f"ek{dc}",
                              tag=f"ek{dc}")
                nc.sync.dma_start(t[:], embkT_in[dc * 128:(dc + 1) * 128, :])
                embk_sb.append(t)
            for dc in range(8):
                t = wsb.tile([128, D], bf16, name=f"wv{dc}", tag=f"wv{dc}")
                nc.sync.dma_start(t[:], wvT_in[dc * 128:(dc + 1) * 128, :])
                wv_t.append(t)

            # ---------------- MT = Wk^T @ Wq  [d', d] ----------------
            # scores = emb_q M emb_k^T with M[d,d'] = sum_e Wq[e,d] Wk[e,d'];
            # we materialize M^T (tiles [128d', 1024d]) as the lhsT source for
            # the K-side fold below.  The e-contraction loop is OUTER with 8
            # bank-aligned accumulators live at once, so the matmuls start as
            # soon as the first wk/wq e-chunk pair lands and stay paced with
            # the input DMA instead of waiting for the full 4MB.
            mt_sb = [wsb.tile([128, D], bf16, name=f"mt{ac}", tag=f"mt{ac}")
                     for ac in range(8)]
            with tc.tile_pool(name="pm4", bufs=4, space="PSUM") as pm4:
                for bb in range(2):
                    t0 = pmm.tile([128, 1024], f32, name=f"pma{bb}", tag="mm")
                    t1 = pmm.tile([128, 1024], f32, name=f"pmb{bb}", tag="mm")
                    q4 = [pm4.tile([128, 512], f32, name=f"pm4_{bb}_{i}",
                                   tag="m4") for i in range(4)]
                    accs = [t0[:, 0:512], t0[:, 512:1024],
                            t1[:, 0:512], t1[:, 512:1024],
                            q4[0][:], q4[1][:], q4[2][:], q4[3][:]]
                    for ec in range(8):
                        for ac in range(8):
                            nc.tensor.matmul(
                                accs[ac],
                                wk_n[ec][:, ac * 128:(ac + 1) * 128],
                                wq_n[ec][:, bb * 512:(bb + 1) * 512],
                                start=(ec == 0), stop=(ec == 7))
                    for ac in range(8):
                        nc.scalar.copy(mt_sb[ac][:, bb * 512:(bb + 1) * 512],
                                       accs[ac])

            # ---------------- V projection (my k rows), SBUF resident -------
            v_sb = []
            for sr in range(16):
                psum = pmm.tile([128, 1024], f32, name=f"pv{sr}", tag="mm")
                col = sr * 128
                for eb in range(2):
                    for dc in range(8):
                        nc.tensor.matmul(
                            psum[:, eb * 512:(eb + 1) * 512],
                            embk_sb[dc][:, col:col + 128],
                            wv_t[dc][:, eb * 512:(eb + 1) * 512],
                            start=(dc == 0), stop=(dc == 7))
                t = vsb.tile([128, 1024], bf16, name=f"v{sr}", tag=f"v{sr}")
                nc.scalar.copy(t[:], psum[:])
                v_sb.append(t)

            # ---------------- KP = M @ embk^T  [d, k] (scores lhsT) ---------
            # KP[d,k] = sum_d' MT[d',d] embk^T[d',k].  16 half-tiles
            # [128d, 1024k]; kh=0 (k tiles 0..7) first so early slots can
            # start while kh=1 computes.  Aliases the dead wk and wv buffers.
            kp_sb = [None] * 16
            for kh in range(2):
                for dc in range(8):
                    psum = pmm.tile([128, 1024], f32, name=f"pk{kh}_{dc}",
                                    tag="mm")
                    for kb in range(2):
                        koff = kh * 1024 + kb * 512
                        for ec in range(8):
                            nc.tensor.matmul(
                                psum[:, kb * 512:(kb + 1) * 512],
                                mt_sb[ec][:, dc * 128:(dc + 1) * 128],
                                embk_sb[ec][:, koff:koff + 512],
                                start=(ec == 0), stop=(ec == 7))
                    alias = f"wk{dc}" if kh == 0 else f"wv{dc}"
                    t = wsb.tile([128, 1024], bf16, name=f"kp{kh}_{dc}",
                                 tag=alias)
                    # kh=1 copies trail into the attention phase; keep them
                    # off the scalar engine so the first slots' exp calls
                    # aren't queued behind them.
                    if kh == 0:
                        nc.scalar.copy(t[:], psum[:])
                    else:
                        nc.vector.tensor_copy(t[:], psum[:])
                    kp_sb[dc * 2 + kh] = t

            # ---------------- attention ----------------
            # slot j = query chunk j (rows 256j..256j+255); k tiles 0..j of
            # this core's half; tile kt lives in kp_sb[dc*2 + kt//8] at column
            # block (kt%8)*128 and v_sb[kt].  Only kt==j is diagonal-masked.
            #
            # Software-pipelined one k-tile deep AND one slot deep: each AV
            # group is emitted after the next scores+exp issue, so the tensor
            # engine never waits on the scalar exp — including at slot
            # boundaries, where the previous slot's last AV group (and its
            # output staging) runs after the next slot's first scores.
            # scores/l PSUM pools open only now: during the M phase their
            # banks were lent to the 8-accumulator pm4 scope.
            _ps_cm = tc.tile_pool(name="ps", bufs=2, space="PSUM")
            _pl_cm = tc.tile_pool(name="pl", bufs=1, space="PSUM")
            ps_pool = _ps_cm.__enter__()
            pl_pool = _pl_cm.__enter__()

            mk_all = mks.tile([128, NSLOT * CHUNK], bf16, name="mka",
                              tag="mka")
            nc.sync.dma_start(mk_all[:], masks_in[:, :])

            def emit_av(l_ps, av, j, wt, kt):
                first, last = kt == 0, kt == j
                vt = v_sb[kt]
                for qs in range(2):
                    wslice = wt[:, qs * 128:(qs + 1) * 128]
                    nc.tensor.matmul(l_ps[qs][:], wslice, ones[:],
                                     start=first, stop=last)
                    for eb in range(2):
                        nc.tensor.matmul(
                            av[qs][:, eb * 512:(eb + 1) * 512], wslice,
                            vt[:, eb * 512:(eb + 1) * 512],
                            start=first, stop=last)

            def finish_slot(prev):
                j, l_ps, av, wt, kt = prev
                emit_av(l_ps, av, j, wt, kt)
                # unnormalized partials; the host divides by the pair-summed
                # denominator.  (DMA can't source PSUM, so stage via SBUF.)
                l_sb = smalls.tile([128, 2], f32, name=f"ls{j}", tag="lst")
                for qs in range(2):
                    nc.vector.tensor_copy(l_sb[:, qs:qs + 1], l_ps[qs][:])
                nc.gpsimd.dma_start(ol_d[j, :, :], l_sb[:])
                for qs in range(2):
                    o_sb = outs.tile([128, 1024], f32, name=f"o{j}_{qs}",
                                     tag="outs")
                    nc.vector.tensor_copy(o_sb[:], av[qs][:])
                    row = (j * 2 + qs) * 128
                    nc.gpsimd.dma_start(oav_d[row:row + 128, :], o_sb[:])

            def scores_exp(eq, j, kt):
                s_ps = ps_pool.tile([128, CHUNK], f32, name=f"s{j}_{kt}",
                                    tag="s")
                half = kt // 8
                kcol = (kt % 8) * 128
                for dc in range(8):
                    nc.tensor.matmul(
                        s_ps[:], kp_sb[dc * 2 + half][:, kcol:kcol + 128],
                        eq[dc][:], start=(dc == 0), stop=(dc == 7))
                wt = wts.tile([128, CHUNK], bf16, name=f"w{j}_{kt}",
                              tag="wts")
                nc.scalar.activation(wt[:], s_ps[:], Exp, bias=0.0,
                                     scale=INV_SQRT_D)
                if kt == j:
                    nc.vector.tensor_mul(
                        wt[:], wt[:], mk_all[:, j * CHUNK:(j + 1) * CHUNK])
                return wt

            prev = None
            for j in range(NSLOT):
                eq = []
                for dc in range(8):
                    t = eqs.tile([128, CHUNK], bf16, name=f"eq{j}_{dc}",
                                 tag="eqs")
                    nc.sync.dma_start(
                        t[:], embT_in[dc * 128:(dc + 1) * 128,
                                      j * CHUNK:(j + 1) * CHUNK])
                    eq.append(t)

                wt0 = scores_exp(eq, j, 0)
                if prev is not None:
                    finish_slot(prev)

                l_ps = [pl_pool.tile([128, 1], f32, name=f"l{j}_{qs}",
                                     tag=f"l{qs}") for qs in range(2)]
                av = [pmm.tile([128, 1024], f32, name=f"av{j}_{qs}", tag="mm")
                      for qs in range(2)]

                pend = (wt0, 0)
                for kt in range(1, j + 1):
                    wt = scores_exp(eq, j, kt)
                    emit_av(l_ps, av, j, *pend)
                    pend = (wt, kt)
                prev = (j, l_ps, av) + pend
            finish_slot(prev)

            _pl_cm.__exit__(None, None, None)
            _ps_cm.__exit__(None, None, None)

    return nc


_CACHED = {}


def _get_graph():
    if "nc" not in _CACHED:
        _install_patches()
        _CACHED["nc"] = _build_graph()
    return _CACHED["nc"]


# ---------------------------------------------------------------------------
# Host-side staging
# ---------------------------------------------------------------------------

def _masks(parity):
    m = np.zeros((NSLOT, 128, CHUNK), dtype=np.float32)
    for j in range(NSLOT):
        p = np.arange(128)[:, None]
        x = np.arange(CHUNK)[None, :]
        m[j] = ((j * CHUNK + x) >= ((2 * j + parity) * 128 + p))
    # device layout: [128 k-partitions, slot-major free dim]
    return np.ascontiguousarray(
        m.transpose(1, 0, 2).reshape(128, NSLOT * CHUNK)).astype(BF16)


def kernel(embeddings, Wq, Wk, Wv):
    embeddings = np.asarray(embeddings, dtype=np.float32)
    Wq = np.asarray(Wq, dtype=np.float32)
    Wk = np.asarray(Wk, dtype=np.float32)
    Wv = np.asarray(Wv, dtype=np.float32)

    nc = _get_graph()
    from concourse.bass_utils import run_bass_kernel_spmd

    wqn = Wq.astype(BF16)
    wkn = Wk.astype(BF16)
    wvT = np.ascontiguousarray(Wv.T).astype(BF16)
    masks_by_par = [_masks(0), _masks(1)]

    in_maps = []
    for c in range(NCORES):
        b, par = divmod(c, 2)
        emb_b = embeddings[b]
        embT = np.ascontiguousarray(emb_b.T).astype(BF16)
        # my k rows: interleaved 128-row tiles (2t+par for t in 0..15)
        embk = np.concatenate(
            [emb_b[(2 * t + par) * 128:(2 * t + par) * 128 + 128]
             for t in range(16)], axis=0)
        embkT = np.ascontiguousarray(embk.T).astype(BF16)
        in_maps.append({
            "embT": embT,
            "embkT": embkT,
            "wqn": wqn,
            "wkn": wkn,
            "wvT": wvT,
            "masks": masks_by_par[par],
        })

    trace = bool(int(os.environ.get("BASS_KERNEL_TRACE", "0")))
    kwargs = {}
    if trace:
        kwargs["trace"] = _install_ntff_hook()

    res = run_bass_kernel_spmd(nc, in_maps, core_ids=list(range(NCORES)),
                               **kwargs)
    _CACHED["last_result"] = res

    out = np.empty((B, S, D), dtype=np.float32)
    for b in range(B):
        r0, r1 = res.results[2 * b], res.results[2 * b + 1]
        av = r0["oav"] + r1["oav"]                      # [S, D]
        l = (r0["ol"] + r1["ol"])                       # [16, 128, 2]
        lfull = l.transpose(0, 2, 1).reshape(S, 1)      # q = 256j+128qs+p
        out[b] = av / lfull
    return out


# revision 6
# speedup vs baseline: 1.0707x; 1.0025x over previous
"""Self-contained Trainium2 (Bass/Tile) kernel: single-head causal attention.

Problem: embeddings [4,4096,1024] f32; Wq/Wk/Wv [1024,1024] f32 (torch Linear
layout [out,in]).  out = softmax(causal(QK^T)/sqrt(D)) @ V, computed per batch.

Distribution (v3): 8 NeuronCores, one SPMD program, context-parallel split.
Core c handles batch c//2; the two cores of a batch pair split the KEY axis by
interleaved 128-row k-tiles (even core: true tiles 0,2,4,...; odd: 1,3,5,...).
Each core processes ALL 4096 query rows against its 2048 k-rows and emits
unnormalized partial attention (sum of exp-weights times V) plus the partial
softmax denominators; the host unshard step adds the pair's partials and
divides.  This is the standard sequence/context-parallel attention combine.

Per-core work: slot j = query chunk j (256 rows) needs exactly j+1 of this
core's k-tiles (perfect causal balance; only each slot's last tile is
diagonal-masked, via a per-core mask table input).  V is projected only for
this core's k-rows (no duplicate work in the pair) and stays resident in
SBUF.  scores = emb_q M emb_k^T with M = Wq^T Wk folded on the K side:
KP = M @ embk^T, so neither Q nor K is ever materialized.

Host-side staging: transpose + bf16-cast, k-tile gather for embk, mask table,
and the final pair combine (add partials, divide by summed denominator).
All matmuls (M, V, KP, scores, AV) and the exp run on device in bf16 with f32
accumulation.
"""

import math
import os
import sys
import types

import numpy as np
import ml_dtypes

B, S, D = 4, 4096, 1024
NCORES = 8
NSLOT = 16
CHUNK = 256          # q rows per slot
KHALF = S // 2       # k rows owned per core
INV_SQRT_D = 1.0 / math.sqrt(D)
BF16 = ml_dtypes.bfloat16


# ---------------------------------------------------------------------------
# Environment patches (compiler workarounds + profiling hook shim)
# ---------------------------------------------------------------------------

def _install_patches():
    import json as _json
    import concourse.bass as bass

    if not getattr(bass.Bass, "_mw_patched", False):
        _orig_to_json = bass.Bass.to_json_bytes

        def to_json_bytes(self):
            # This walrus build rejects any instruction carrying more than one
            # sync wait ("Too many sync wait commands").  Split extra waits
            # onto single-wait NoOps inserted just before the instruction (the
            # engine executes them in order, so semantics are unchanged).
            raw = _orig_to_json(self)
            m = _json.loads(raw)
            ctr = 0
            changed = False
            for fn in m.get("functions", []):
                for bb in fn.get("blocks", []):
                    out = []
                    for inst in bb.get("instructions", []):
                        si = inst.get("sync_info")
                        if si:
                            waits = si.get("on_wait") or []
                            if len(waits) > 1:
                                changed = True
                                for w in waits[:-1]:
                                    ctr += 1
                                    out.append({
                                        "debug": inst.get("debug", 0),
                                        "engine": inst["engine"],
                                        "ins": [],
                                        "outs": [],
                                        "name": f"I-mw{ctr}",
                                        "opcode": "NoOp",
                                        "text_hint": "mwsplit",
                                        "sync_info": {"on_wait": [w],
                                                      "on_update": []},
                                    })
                                si["on_wait"] = [waits[-1]]
                        out.append(inst)
                    bb["instructions"] = out
            if not changed:
                return raw
            return _json.dumps(m).encode()

        bass.Bass.to_json_bytes = to_json_bytes
        bass.Bass._mw_patched = True

    # Don't upload NEFF/trace artifacts anywhere; keep them local.
    import concourse.bass_utils as bu
    bu.upload_artifacts = lambda tmpdir: tmpdir


def _install_ntff_hook() -> bool:
    """Register the axon NTFF profiling hook (missing module in this image)."""
    try:
        import antenv.axon_hooks  # noqa: F401
        return True
    except ImportError:
        pass
    try:
        mod = types.ModuleType("antenv.axon_hooks")
        state = {"hook": None}
        mod.set_axon_ntff_profile_hook = lambda h: state.__setitem__("hook", h)
        mod.get_axon_ntff_profile_hook = lambda: state["hook"]
        sys.modules["antenv.axon_hooks"] = mod
        import antenv
        antenv.axon_hooks = mod
        from trn_agent_boot.trn_boot import _ntff_profile_via_ctypes
        mod.set_axon_ntff_profile_hook(
            _ntff_profile_via_ctypes("/opt/axon/libaxon_pjrt.so"))
        return True
    except Exception:
        return False


# ---------------------------------------------------------------------------
# Graph
# ---------------------------------------------------------------------------

def _build_graph():
    import concourse.bass as bass
    import concourse.bass_isa as bass_isa
    import concourse.mybir as mybir
    import concourse.tile as tile

    f32 = mybir.dt.float32
    bf16 = mybir.dt.bfloat16
    Exp = mybir.ActivationFunctionType.Exp

    nc = bass.Bass("TRN2", debug=False, num_devices=NCORES)

    embT_in = nc.dram_tensor("embT", [D, S], bf16, kind="ExternalInput")
    embkT_in = nc.dram_tensor("embkT", [D, KHALF], bf16, kind="ExternalInput")
    wq_in = nc.dram_tensor("wqn", [D, D], bf16, kind="ExternalInput")
    wk_in = nc.dram_tensor("wkn", [D, D], bf16, kind="ExternalInput")
    wvT_in = nc.dram_tensor("wvT", [D, D], bf16, kind="ExternalInput")
    masks_in = nc.dram_tensor("masks", [128, NSLOT * CHUNK], bf16,
                              kind="ExternalInput")
    oav_d = nc.dram_tensor("oav", [S, D], f32, kind="ExternalOutput")
    ol_d = nc.dram_tensor("ol", [NSLOT, 128, CHUNK], f32,
                          kind="ExternalOutput")

    with tile.TileContext(nc) as tc:
        with (
            tc.tile_pool(name="wsb", bufs=1) as wsb,          # weights resident
            tc.tile_pool(name="eksb", bufs=1) as eksb,        # embk^T resident
            tc.tile_pool(name="vsb", bufs=1) as vsb,          # V resident
            tc.tile_pool(name="eqs", bufs=16) as eqs,         # embT q-side stream
            tc.tile_pool(name="mks", bufs=1) as mks,          # masks resident
            tc.tile_pool(name="wts", bufs=6) as wts,          # exp weights
            tc.tile_pool(name="outs", bufs=3) as outs,        # av out stage
            tc.tile_pool(name="lacc", bufs=2) as lacc,        # exp-sum accum
            tc.tile_pool(name="pmm", bufs=2, space="PSUM") as pmm,
        ):
            # resident weight tiles; DMA order = need order (M first)
            wk_n, wq_n, wv_t = [], [], []
            for dc in range(8):
                t = wsb.tile([128, D], bf16, name=f"wk{dc}", tag=f"wk{dc}")
                nc.sync.dma_start(t[:], wk_in[dc * 128:(dc + 1) * 128, :])
                wk_n.append(t)
                t = wsb.tile([128, D], bf16, name=f"wq{dc}", tag=f"wq{dc}")
                nc.sync.dma_start(t[:], wq_in[dc * 128:(dc + 1) * 128, :])
                wq_n.append(t)
            embk_sb = []
            for dc in range(8):
                t = eksb.tile([128, KHALF], bf16, name=f"ek{dc}",
                              tag=f"ek{dc}")
                nc.sync.dma_start(t[:], embkT_in[dc * 128:(dc + 1) * 128, :])
                embk_sb.append(t)
            for dc in range(8):
                t = wsb.tile([128, D], bf16, name=f"wv{dc}", tag=f"wv{dc}")
                nc.sync.dma_start(t[:], wvT_in[dc * 128:(dc + 1) * 128, :])
                wv_t.append(t)

            # ---------------- MT = Wk^T @ Wq  [d', d] ----------------
            # scores = emb_q M emb_k^T with M[d,d'] = sum_e Wq[e,d] Wk[e,d'];
            # we materialize M^T (tiles [128d', 1024d]) as the lhsT source for
            # the K-side fold below.  The e-contraction loop is OUTER with 8
            # bank-aligned accumulators live at once, so the matmuls start as
            # soon as the first wk/wq e-chunk pair lands and stay paced with
            # the input DMA instead of waiting for the full 4MB.
            mt_sb = [wsb.tile([128, D], bf16, name=f"mt{ac}", tag=f"mt{ac}")
                     for ac in range(8)]
            with tc.tile_pool(name="pm4", bufs=4, space="PSUM") as pm4:
                for bb in range(2):
                    t0 = pmm.tile([128, 1024], f32, name=f"pma{bb}", tag="mm")
                    t1 = pmm.tile([128, 1024], f32, name=f"pmb{bb}", tag="mm")
                    q4 = [pm4.tile([128, 512], f32, name=f"pm4_{bb}_{i}",
                                   tag="m4") for i in range(4)]
                    accs = [t0[:, 0:512], t0[:, 512:1024],
                            t1[:, 0:512], t1[:, 512:1024],
                            q4[0][:], q4[1][:], q4[2][:], q4[3][:]]
                    for ec in range(8):
                        for ac in range(8):
                            nc.tensor.matmul(
                                accs[ac],
                                wk_n[ec][:, ac * 128:(ac + 1) * 128],
                                wq_n[ec][:, bb * 512:(bb + 1) * 512],
                                start=(ec == 0), stop=(ec == 7))
                    for ac in range(8):
                        nc.scalar.copy(mt_sb[ac][:, bb * 512:(bb + 1) * 512],
                                       accs[ac])

            # ---------------- V projection (my k rows), SBUF resident -------
            v_sb = []
            for sr in range(16):
                psum = pmm.tile([128, 1024], f32, name=f"pv{sr}", tag="mm")
                col = sr * 128
                for eb in range(2):
                    for dc in range(8):
                        nc.tensor.matmul(
                            psum[:, eb * 512:(eb + 1) * 512],
                            embk_sb[dc][:, col:col + 128],
                            wv_t[dc][:, eb * 512:(eb + 1) * 512],
                            start=(dc == 0), stop=(dc == 7))
                t = vsb.tile([128, 1024], bf16, name=f"v{sr}", tag=f"v{sr}")
                nc.scalar.copy(t[:], psum[:])
                v_sb.append(t)

            # ---------------- KP = M @ embk^T  [d, k] (scores lhsT) ---------
            # KP[d,k] = sum_d' MT[d',d] embk^T[d',k].  16 half-tiles
            # [128d, 1024k]; kh=0 (k tiles 0..7) first so early slots can
            # start while kh=1 computes.  Aliases the dead wk and wv buffers.
            kp_sb = [None] * 16
            for kh in range(2):
                for dc in range(8):
                    psum = pmm.tile([128, 1024], f32, name=f"pk{kh}_{dc}",
                                    tag="mm")
                    for kb in range(2):
                        koff = kh * 1024 + kb * 512
                        for ec in range(8):
                            nc.tensor.matmul(
                                psum[:, kb * 512:(kb + 1) * 512],
                                mt_sb[ec][:, dc * 128:(dc + 1) * 128],
                                embk_sb[ec][:, koff:koff + 512],
                                start=(ec == 0), stop=(ec == 7))
                    alias = f"wk{dc}" if kh == 0 else f"wv{dc}"
                    t = wsb.tile([128, 1024], bf16, name=f"kp{kh}_{dc}",
                                 tag=alias)
                    # kh=1 copies trail into the attention phase; keep them
                    # off the scalar engine so the first slots' exp calls
                    # aren't queued behind them.
                    if kh == 0:
                        nc.scalar.copy(t[:], psum[:])
                    else:
                        nc.vector.tensor_copy(t[:], psum[:])
                    kp_sb[dc * 2 + kh] = t

            # ---------------- attention ----------------
            # slot j = query chunk j (rows 256j..256j+255); k tiles 0..j of
            # this core's half; tile kt lives in kp_sb[dc*2 + kt//8] at column
            # block (kt%8)*128 and v_sb[kt].  Only kt==j is diagonal-masked.
            #
            # Software-pipelined one k-tile deep AND one slot deep: each AV
            # group is emitted after the next scores+exp issue, so the tensor
            # engine never waits on the scalar exp — including at slot
            # boundaries, where the previous slot's last AV group (and its
            # output staging) runs after the next slot's first scores.
            # scores PSUM pool opens only now: during the M phase its banks
            # were lent to the 8-accumulator pm4 scope.  The softmax
            # denominators never touch PSUM (vector-summed in SBUF, folded on
            # the host), so the freed banks buy a third scores buffer.
            _ps_cm = tc.tile_pool(name="ps", bufs=3, space="PSUM")
            ps_pool = _ps_cm.__enter__()

            mk_all = mks.tile([128, NSLOT * CHUNK], bf16, name="mka",
                              tag="mka")
            nc.sync.dma_start(mk_all[:], masks_in[:, :])

            def emit_av(av, j, wt, kt):
                first, last = kt == 0, kt == j
                vt = v_sb[kt]
                for qs in range(2):
                    wslice = wt[:, qs * 128:(qs + 1) * 128]
                    for eb in range(2):
                        nc.tensor.matmul(
                            av[qs][:, eb * 512:(eb + 1) * 512], wslice,
                            vt[:, eb * 512:(eb + 1) * 512],
                            start=first, stop=last)

            def finish_slot(prev):
                j, w_acc, av, wt, kt = prev
                emit_av(av, j, wt, kt)
                # softmax denominator: exp-weights were elementwise-summed
                # over k-tiles on vector (f32).  Ship the [128, 256] partial
                # tile; the host folds the 128 partition lanes together with
                # the pair combine.
                nc.gpsimd.dma_start(ol_d[j, :, :], w_acc[:])
                for qs in range(2):
                    o_sb = outs.tile([128, 1024], f32, name=f"o{j}_{qs}",
                                     tag="outs")
                    nc.vector.tensor_copy(o_sb[:], av[qs][:])
                    row = (j * 2 + qs) * 128
                    nc.gpsimd.dma_start(oav_d[row:row + 128, :], o_sb[:])

            def scores_exp(eq, j, kt, w_acc):
                s_ps = ps_pool.tile([128, CHUNK], f32, name=f"s{j}_{kt}",
                                    tag="s")
                half = kt // 8
                kcol = (kt % 8) * 128
                for dc in range(8):
                    nc.tensor.matmul(
                        s_ps[:], kp_sb[dc * 2 + half][:, kcol:kcol + 128],
                        eq[dc][:], start=(dc == 0), stop=(dc == 7))
                wt = wts.tile([128, CHUNK], bf16, name=f"w{j}_{kt}",
                              tag="wts")
                nc.scalar.activation(wt[:], s_ps[:], Exp, bias=0.0,
                                     scale=INV_SQRT_D)
                if kt == j:
                    nc.vector.tensor_mul(
                        wt[:], wt[:], mk_all[:, j * CHUNK:(j + 1) * CHUNK])
                if kt == 0:
                    nc.vector.tensor_copy(w_acc[:], wt[:])
                else:
                    nc.vector.tensor_add(w_acc[:], w_acc[:], wt[:])
                return wt

            prev = None
            for j in range(NSLOT):
                eq = []
                for dc in range(8):
                    t = eqs.tile([128, CHUNK], bf16, name=f"eq{j}_{dc}",
                                 tag="eqs")
                    nc.sync.dma_start(
                        t[:], embT_in[dc * 128:(dc + 1) * 128,
                                      j * CHUNK:(j + 1) * CHUNK])
                    eq.append(t)

                w_acc = lacc.tile([128, CHUNK], f32, name=f"wa{j}", tag="wacc")
                wt0 = scores_exp(eq, j, 0, w_acc)
                if prev is not None:
                    finish_slot(prev)

                av = [pmm.tile([128, 1024], f32, name=f"av{j}_{qs}", tag="mm")
                      for qs in range(2)]

                pend = (wt0, 0)
                for kt in range(1, j + 1):
                    wt = scores_exp(eq, j, kt, w_acc)
                    emit_av(av, j, *pend)
                    pend = (wt, kt)
                prev = (j, w_acc, av) + pend
            finish_slot(prev)

            _ps_cm.__exit__(None, None, None)

    return nc


_CACHED = {}


def _get_graph():
    if "nc" not in _CACHED:
        _install_patches()
        _CACHED["nc"] = _build_graph()
    return _CACHED["nc"]


# ---------------------------------------------------------------------------
# Host-side staging
# ---------------------------------------------------------------------------

def _masks(parity):
    m = np.zeros((NSLOT, 128, CHUNK), dtype=np.float32)
    for j in range(NSLOT):
        p = np.arange(128)[:, None]
        x = np.arange(CHUNK)[None, :]
        m[j] = ((j * CHUNK + x) >= ((2 * j + parity) * 128 + p))
    # device layout: [128 k-partitions, slot-major free dim]
    return np.ascontiguousarray(
        m.transpose(1, 0, 2).reshape(128, NSLOT * CHUNK)).astype(BF16)


def kernel(embeddings, Wq, Wk, Wv):
    embeddings = np.asarray(embeddings, dtype=np.float32)
    Wq = np.asarray(Wq, dtype=np.float32)
    Wk = np.asarray(Wk, dtype=np.float32)
    Wv = np.asarray(Wv, dtype=np.float32)

    nc = _get_graph()
    from concourse.bass_utils import run_bass_kernel_spmd

    wqn = Wq.astype(BF16)
    wkn = Wk.astype(BF16)
    wvT = np.ascontiguousarray(Wv.T).astype(BF16)
    masks_by_par = [_masks(0), _masks(1)]

    in_maps = []
    for c in range(NCORES):
        b, par = divmod(c, 2)
        emb_b = embeddings[b]
        embT = np.ascontiguousarray(emb_b.T).astype(BF16)
        # my k rows: interleaved 128-row tiles (2t+par for t in 0..15)
        embk = np.concatenate(
            [emb_b[(2 * t + par) * 128:(2 * t + par) * 128 + 128]
             for t in range(16)], axis=0)
        embkT = np.ascontiguousarray(embk.T).astype(BF16)
        in_maps.append({
            "embT": embT,
            "embkT": embkT,
            "wqn": wqn,
            "wkn": wkn,
            "wvT": wvT,
            "masks": masks_by_par[par],
        })

    trace = bool(int(os.environ.get("BASS_KERNEL_TRACE", "0")))
    kwargs = {}
    if trace:
        kwargs["trace"] = _install_ntff_hook()

    res = run_bass_kernel_spmd(nc, in_maps, core_ids=list(range(NCORES)),
                               **kwargs)
    _CACHED["last_result"] = res

    out = np.empty((B, S, D), dtype=np.float32)
    for b in range(B):
        r0, r1 = res.results[2 * b], res.results[2 * b + 1]
        av = r0["oav"] + r1["oav"]                      # [S, D]
        # fold the per-core k-partition lanes and the pair halves
        l = (r0["ol"] + r1["ol"]).sum(axis=1)           # [16, 256], q-major
        out[b] = av / l.reshape(S, 1)
    return out


# revision 7
# speedup vs baseline: 1.0865x; 1.0148x over previous
"""Self-contained Trainium2 (Bass/Tile) kernel: single-head causal attention.

Problem: embeddings [4,4096,1024] f32; Wq/Wk/Wv [1024,1024] f32 (torch Linear
layout [out,in]).  out = softmax(causal(QK^T)/sqrt(D)) @ V, computed per batch.

Distribution (v3): 8 NeuronCores, one SPMD program, context-parallel split.
Core c handles batch c//2; the two cores of a batch pair split the KEY axis by
interleaved 128-row k-tiles (even core: true tiles 0,2,4,...; odd: 1,3,5,...).
Each core processes ALL 4096 query rows against its 2048 k-rows and emits
unnormalized partial attention (sum of exp-weights times V) plus the partial
softmax denominators; the host unshard step adds the pair's partials and
divides.  This is the standard sequence/context-parallel attention combine.

Per-core work: slot j = query chunk j (256 rows) needs exactly j+1 of this
core's k-tiles (perfect causal balance; only each slot's last tile is
diagonal-masked, via a per-core mask table input).  V is projected only for
this core's k-rows (no duplicate work in the pair) and stays resident in
SBUF.  scores = emb_q M emb_k^T with M = Wq^T Wk folded on the K side:
KP = M @ embk^T, so neither Q nor K is ever materialized.

Host-side staging: transpose + bf16-cast, k-tile gather for embk, mask table,
and the final pair combine (add partials, divide by summed denominator).
All matmuls (M, V, KP, scores, AV) and the exp run on device in bf16 with f32
accumulation.
"""

import math
import os
import sys
import types

import numpy as np
import ml_dtypes

B, S, D = 4, 4096, 1024
NCORES = 8
NSLOT = 16
CHUNK = 256          # q rows per slot
KHALF = S // 2       # k rows owned per core
INV_SQRT_D = 1.0 / math.sqrt(D)
BF16 = ml_dtypes.bfloat16


# ---------------------------------------------------------------------------
# Environment patches (compiler workarounds + profiling hook shim)
# ---------------------------------------------------------------------------

def _install_patches():
    import json as _json
    import concourse.bass as bass

    if not getattr(bass.Bass, "_mw_patched", False):
        _orig_to_json = bass.Bass.to_json_bytes

        def to_json_bytes(self):
            # This walrus build rejects any instruction carrying more than one
            # sync wait ("Too many sync wait commands").  Split extra waits
            # onto single-wait NoOps inserted just before the instruction (the
            # engine executes them in order, so semantics are unchanged).
            raw = _orig_to_json(self)
            m = _json.loads(raw)
            ctr = 0
            changed = False
            for fn in m.get("functions", []):
                for bb in fn.get("blocks", []):
                    out = []
                    for inst in bb.get("instructions", []):
                        si = inst.get("sync_info")
                        if si:
                            waits = si.get("on_wait") or []
                            if len(waits) > 1:
                                changed = True
                                for w in waits[:-1]:
                                    ctr += 1
                                    out.append({
                                        "debug": inst.get("debug", 0),
                                        "engine": inst["engine"],
                                        "ins": [],
                                        "outs": [],
                                        "name": f"I-mw{ctr}",
                                        "opcode": "NoOp",
                                        "text_hint": "mwsplit",
                                        "sync_info": {"on_wait": [w],
                                                      "on_update": []},
                                    })
                                si["on_wait"] = [waits[-1]]
                        out.append(inst)
                    bb["instructions"] = out
            if not changed:
                return raw
            return _json.dumps(m).encode()

        bass.Bass.to_json_bytes = to_json_bytes
        bass.Bass._mw_patched = True

    # Don't upload NEFF/trace artifacts anywhere; keep them local.
    import concourse.bass_utils as bu
    bu.upload_artifacts = lambda tmpdir: tmpdir


def _install_ntff_hook() -> bool:
    """Register the axon NTFF profiling hook (missing module in this image)."""
    try:
        import antenv.axon_hooks  # noqa: F401
        return True
    except ImportError:
        pass
    try:
        mod = types.ModuleType("antenv.axon_hooks")
        state = {"hook": None}
        mod.set_axon_ntff_profile_hook = lambda h: state.__setitem__("hook", h)
        mod.get_axon_ntff_profile_hook = lambda: state["hook"]
        sys.modules["antenv.axon_hooks"] = mod
        import antenv
        antenv.axon_hooks = mod
        from trn_agent_boot.trn_boot import _ntff_profile_via_ctypes
        mod.set_axon_ntff_profile_hook(
            _ntff_profile_via_ctypes("/opt/axon/libaxon_pjrt.so"))
        return True
    except Exception:
        return False


# ---------------------------------------------------------------------------
# Graph
# ---------------------------------------------------------------------------

def _build_graph():
    import concourse.bass as bass
    import concourse.bass_isa as bass_isa
    import concourse.mybir as mybir
    import concourse.tile as tile

    f32 = mybir.dt.float32
    bf16 = mybir.dt.bfloat16
    Exp = mybir.ActivationFunctionType.Exp

    nc = bass.Bass("TRN2", debug=False, num_devices=NCORES)

    embT_in = nc.dram_tensor("embT", [D, S], bf16, kind="ExternalInput")
    embkT_in = nc.dram_tensor("embkT", [D, KHALF], bf16, kind="ExternalInput")
    wkq_in = nc.dram_tensor("wkq", [D, 2 * D], bf16, kind="ExternalInput")
    wvT_in = nc.dram_tensor("wvT", [D, D], bf16, kind="ExternalInput")
    masks_in = nc.dram_tensor("masks", [128, NSLOT * CHUNK], bf16,
                              kind="ExternalInput")
    oav_d = nc.dram_tensor("oav", [S, D], f32, kind="ExternalOutput")
    ol_d = nc.dram_tensor("ol", [NSLOT, 128, CHUNK], f32,
                          kind="ExternalOutput")

    with tile.TileContext(nc) as tc:
        with (
            tc.tile_pool(name="wsb", bufs=1) as wsb,          # weights resident
            tc.tile_pool(name="eksb", bufs=1) as eksb,        # embk^T resident
            tc.tile_pool(name="vsb", bufs=1) as vsb,          # V resident
            tc.tile_pool(name="eqs", bufs=16) as eqs,         # embT q-side stream
            tc.tile_pool(name="mks", bufs=1) as mks,          # masks resident
            tc.tile_pool(name="wts", bufs=6) as wts,          # exp weights
            tc.tile_pool(name="outs", bufs=3) as outs,        # av out stage
            tc.tile_pool(name="lacc", bufs=2) as lacc,        # exp-sum accum
            tc.tile_pool(name="pmm", bufs=3, space="PSUM") as pmm,
        ):
            # resident weight tiles; DMA order = need order (M first).
            # wk|wq live side by side in one [128, 2048] tile per e-chunk:
            # half the DMA triggers (trigger issue paces the M phase).
            wkq_sb, wv_t = [], []
            for dc in range(8):
                t = wsb.tile([128, 2 * D], bf16, name=f"wkq{dc}",
                             tag=f"wkq{dc}")
                nc.sync.dma_start(t[:], wkq_in[dc * 128:(dc + 1) * 128, :])
                wkq_sb.append(t)
            embk_sb = []
            for dc in range(8):
                t = eksb.tile([128, KHALF], bf16, name=f"ek{dc}",
                              tag=f"ek{dc}")
                nc.sync.dma_start(t[:], embkT_in[dc * 128:(dc + 1) * 128, :])
                embk_sb.append(t)
            for dc in range(8):
                t = wsb.tile([128, D], bf16, name=f"wv{dc}", tag=f"wv{dc}")
                nc.sync.dma_start(t[:], wvT_in[dc * 128:(dc + 1) * 128, :])
                wv_t.append(t)

            # ---------------- MT = Wk^T @ Wq  [d', d] ----------------
            # scores = emb_q M emb_k^T with M[d,d'] = sum_e Wq[e,d] Wk[e,d'];
            # we materialize M^T (tiles [128d', 1024d]) as the lhsT source for
            # the K-side fold below.  The e-contraction loop is OUTER with 8
            # bank-aligned accumulators live at once, so the matmuls start as
            # soon as the first wk/wq e-chunk pair lands and stay paced with
            # the input DMA instead of waiting for the full 4MB.
            mt_sb = [wsb.tile([128, D], bf16, name=f"mt{ac}", tag=f"mt{ac}")
                     for ac in range(8)]
            with tc.tile_pool(name="pm4", bufs=2, space="PSUM") as pm4:
                for bb in range(2):
                    t3 = [pmm.tile([128, 1024], f32, name=f"pm{bb}_{i}",
                                   tag="mm") for i in range(3)]
                    q4 = [pm4.tile([128, 512], f32, name=f"pm4_{bb}_{i}",
                                   tag="m4") for i in range(2)]
                    accs = [t3[0][:, 0:512], t3[0][:, 512:1024],
                            t3[1][:, 0:512], t3[1][:, 512:1024],
                            t3[2][:, 0:512], t3[2][:, 512:1024],
                            q4[0][:], q4[1][:]]
                    for ec in range(8):
                        for ac in range(8):
                            nc.tensor.matmul(
                                accs[ac],
                                wkq_sb[ec][:, ac * 128:(ac + 1) * 128],
                                wkq_sb[ec][:, D + bb * 512:
                                            D + (bb + 1) * 512],
                                start=(ec == 0), stop=(ec == 7))
                    for ac in range(8):
                        nc.scalar.copy(mt_sb[ac][:, bb * 512:(bb + 1) * 512],
                                       accs[ac])

            # ---------------- V projection (my k rows), SBUF resident -------
            v_sb = []
            for sr in range(16):
                psum = pmm.tile([128, 1024], f32, name=f"pv{sr}", tag="mm")
                col = sr * 128
                for eb in range(2):
                    for dc in range(8):
                        nc.tensor.matmul(
                            psum[:, eb * 512:(eb + 1) * 512],
                            embk_sb[dc][:, col:col + 128],
                            wv_t[dc][:, eb * 512:(eb + 1) * 512],
                            start=(dc == 0), stop=(dc == 7))
                t = vsb.tile([128, 1024], bf16, name=f"v{sr}", tag=f"v{sr}")
                nc.scalar.copy(t[:], psum[:])
                v_sb.append(t)

            # ---------------- KP = M @ embk^T  [d, k] (scores lhsT) ---------
            # KP[d,k] = sum_d' MT[d',d] embk^T[d',k].  One [128d, 2048k]
            # tile per d-chunk, aliasing the dead wkq buffers; kh=0 (k tiles
            # 0..7) computed first.
            kp2 = [wsb.tile([128, 2 * D], bf16, name=f"kp{dc}",
                            tag=f"wkq{dc}") for dc in range(8)]
            for kh in range(2):
                for dc in range(8):
                    psum = pmm.tile([128, 1024], f32, name=f"pk{kh}_{dc}",
                                    tag="mm")
                    for kb in range(2):
                        koff = kh * 1024 + kb * 512
                        for ec in range(8):
                            nc.tensor.matmul(
                                psum[:, kb * 512:(kb + 1) * 512],
                                mt_sb[ec][:, dc * 128:(dc + 1) * 128],
                                embk_sb[ec][:, koff:koff + 512],
                                start=(ec == 0), stop=(ec == 7))
                    # kh=1 copies trail into the attention phase; keep them
                    # off the scalar engine so the first slots' exp calls
                    # aren't queued behind them.
                    dst = kp2[dc][:, kh * 1024:(kh + 1) * 1024]
                    if kh == 0:
                        nc.scalar.copy(dst, psum[:])
                    else:
                        nc.vector.tensor_copy(dst, psum[:])

            # ---------------- attention ----------------
            # slot j = query chunk j (rows 256j..256j+255); k tiles 0..j of
            # this core's half; tile kt lives in kp2[dc] at column block
            # kt*128 and v_sb[kt].  Only kt==j is diagonal-masked.
            #
            # Software-pipelined one k-tile deep AND one slot deep: each AV
            # group is emitted after the next scores+exp issue, so the tensor
            # engine never waits on the scalar exp — including at slot
            # boundaries, where the previous slot's last AV group (and its
            # output staging) runs after the next slot's first scores.
            # scores PSUM pool opens only now: during the M phase its banks
            # were lent to the 8-accumulator pm4 scope.  The softmax
            # denominators never touch PSUM (vector-summed in SBUF, folded on
            # the host), so the freed banks buy a third scores buffer.
            _ps_cm = tc.tile_pool(name="ps", bufs=2, space="PSUM")
            ps_pool = _ps_cm.__enter__()

            mk_all = mks.tile([128, NSLOT * CHUNK], bf16, name="mka",
                              tag="mka")
            nc.sync.dma_start(mk_all[:], masks_in[:, :])

            def emit_av(av, j, wt, kt):
                first, last = kt == 0, kt == j
                vt = v_sb[kt]
                for qs in range(2):
                    wslice = wt[:, qs * 128:(qs + 1) * 128]
                    for eb in range(2):
                        nc.tensor.matmul(
                            av[qs][:, eb * 512:(eb + 1) * 512], wslice,
                            vt[:, eb * 512:(eb + 1) * 512],
                            start=first, stop=last)

            def finish_slot(prev):
                j, w_acc, av, wt, kt = prev
                emit_av(av, j, wt, kt)
                # softmax denominator: exp-weights were elementwise-summed
                # over k-tiles on vector (f32).  Ship the [128, 256] partial
                # tile; the host folds the 128 partition lanes together with
                # the pair combine.
                nc.gpsimd.dma_start(ol_d[j, :, :], w_acc[:])
                for qs in range(2):
                    o_sb = outs.tile([128, 1024], f32, name=f"o{j}_{qs}",
                                     tag="outs")
                    nc.vector.tensor_copy(o_sb[:], av[qs][:])
                    row = (j * 2 + qs) * 128
                    nc.gpsimd.dma_start(oav_d[row:row + 128, :], o_sb[:])

            def scores_exp(eq, j, kt, w_acc):
                s_ps = ps_pool.tile([128, CHUNK], f32, name=f"s{j}_{kt}",
                                    tag="s")
                for dc in range(8):
                    nc.tensor.matmul(
                        s_ps[:], kp2[dc][:, kt * 128:(kt + 1) * 128],
                        eq[dc][:], start=(dc == 0), stop=(dc == 7))
                wt = wts.tile([128, CHUNK], bf16, name=f"w{j}_{kt}",
                              tag="wts")
                nc.scalar.activation(wt[:], s_ps[:], Exp, bias=0.0,
                                     scale=INV_SQRT_D)
                if kt == j:
                    nc.vector.tensor_mul(
                        wt[:], wt[:], mk_all[:, j * CHUNK:(j + 1) * CHUNK])
                if kt == 0:
                    nc.vector.tensor_copy(w_acc[:], wt[:])
                else:
                    nc.vector.tensor_add(w_acc[:], w_acc[:], wt[:])
                return wt

            # slot 7 first: eight dense scores groups warm the
            # pipeline while the KP evacuation copies drain, instead
            # of the thin slots 0-2 (it needs only the kh=0 KP half).
            prev = None
            for j in [7] + [x for x in range(NSLOT) if x != 7]:
                eq = []
                for dc in range(8):
                    t = eqs.tile([128, CHUNK], bf16, name=f"eq{j}_{dc}",
                                 tag="eqs")
                    nc.sync.dma_start(
                        t[:], embT_in[dc * 128:(dc + 1) * 128,
                                      j * CHUNK:(j + 1) * CHUNK])
                    eq.append(t)

                w_acc = lacc.tile([128, CHUNK], f32, name=f"wa{j}", tag="wacc")
                wt0 = scores_exp(eq, j, 0, w_acc)
                if prev is not None:
                    finish_slot(prev)

                av = [pmm.tile([128, 1024], f32, name=f"av{j}_{qs}", tag="mm")
                      for qs in range(2)]

                pend = (wt0, 0)
                for kt in range(1, j + 1):
                    wt = scores_exp(eq, j, kt, w_acc)
                    emit_av(av, j, *pend)
                    pend = (wt, kt)
                prev = (j, w_acc, av) + pend
            finish_slot(prev)

            _ps_cm.__exit__(None, None, None)

    return nc


_CACHED = {}


def _get_graph():
    if "nc" not in _CACHED:
        _install_patches()
        _CACHED["nc"] = _build_graph()
    return _CACHED["nc"]


# ---------------------------------------------------------------------------
# Host-side staging
# ---------------------------------------------------------------------------

def _masks(parity):
    m = np.zeros((NSLOT, 128, CHUNK), dtype=np.float32)
    for j in range(NSLOT):
        p = np.arange(128)[:, None]
        x = np.arange(CHUNK)[None, :]
        m[j] = ((j * CHUNK + x) >= ((2 * j + parity) * 128 + p))
    # device layout: [128 k-partitions, slot-major free dim]
    return np.ascontiguousarray(
        m.transpose(1, 0, 2).reshape(128, NSLOT * CHUNK)).astype(BF16)


def kernel(embeddings, Wq, Wk, Wv):
    embeddings = np.asarray(embeddings, dtype=np.float32)
    Wq = np.asarray(Wq, dtype=np.float32)
    Wk = np.asarray(Wk, dtype=np.float32)
    Wv = np.asarray(Wv, dtype=np.float32)

    nc = _get_graph()
    from concourse.bass_utils import run_bass_kernel_spmd

    wkq = np.ascontiguousarray(
        np.concatenate([Wk, Wq], axis=1)).astype(BF16)
    wvT = np.ascontiguousarray(Wv.T).astype(BF16)
    masks_by_par = [_masks(0), _masks(1)]

    in_maps = []
    for c in range(NCORES):
        b, par = divmod(c, 2)
        emb_b = embeddings[b]
        embT = np.ascontiguousarray(emb_b.T).astype(BF16)
        # my k rows: interleaved 128-row tiles (2t+par for t in 0..15)
        embk = np.concatenate(
            [emb_b[(2 * t + par) * 128:(2 * t + par) * 128 + 128]
             for t in range(16)], axis=0)
        embkT = np.ascontiguousarray(embk.T).astype(BF16)
        in_maps.append({
            "embT": embT,
            "embkT": embkT,
            "wkq": wkq,
            "wvT": wvT,
            "masks": masks_by_par[par],
        })

    trace = bool(int(os.environ.get("BASS_KERNEL_TRACE", "0")))
    kwargs = {}
    if trace:
        kwargs["trace"] = _install_ntff_hook()

    res = run_bass_kernel_spmd(nc, in_maps, core_ids=list(range(NCORES)),
                               **kwargs)
    _CACHED["last_result"] = res

    out = np.empty((B, S, D), dtype=np.float32)
    for b in range(B):
        r0, r1 = res.results[2 * b], res.results[2 * b + 1]
        av = r0["oav"] + r1["oav"]                      # [S, D]
        # fold the per-core k-partition lanes and the pair halves
        l = (r0["ol"] + r1["ol"]).sum(axis=1)           # [16, 256], q-major
        out[b] = av / l.reshape(S, 1)
    return out


# revision 8
# speedup vs baseline: 1.0910x; 1.0041x over previous
"""Self-contained Trainium2 (Bass/Tile) kernel: single-head causal attention.

Problem: embeddings [4,4096,1024] f32; Wq/Wk/Wv [1024,1024] f32 (torch Linear
layout [out,in]).  out = softmax(causal(QK^T)/sqrt(D)) @ V, computed per batch.

Distribution (v3): 8 NeuronCores, one SPMD program, context-parallel split.
Core c handles batch c//2; the two cores of a batch pair split the KEY axis by
interleaved 128-row k-tiles (even core: true tiles 0,2,4,...; odd: 1,3,5,...).
Each core processes ALL 4096 query rows against its 2048 k-rows and emits
unnormalized partial attention (sum of exp-weights times V) plus the partial
softmax denominators; the host unshard step adds the pair's partials and
divides.  This is the standard sequence/context-parallel attention combine.

Per-core work: slot j = query chunk j (256 rows) needs exactly j+1 of this
core's k-tiles (perfect causal balance; only each slot's last tile is
diagonal-masked, via a per-core mask table input).  V is projected only for
this core's k-rows (no duplicate work in the pair) and stays resident in
SBUF.  scores = emb_q M emb_k^T with M = Wq^T Wk folded on the K side:
KP = M @ embk^T, so neither Q nor K is ever materialized.

Host-side staging: transpose + bf16-cast, k-tile gather for embk, mask table,
and the final pair combine (add partials, divide by summed denominator).
All matmuls (M, V, KP, scores, AV) and the exp run on device in bf16 with f32
accumulation.
"""

import math
import os
import sys
import types

import numpy as np
import ml_dtypes

B, S, D = 4, 4096, 1024
NCORES = 8
NSLOT = 16
CHUNK = 256          # q rows per slot
KHALF = S // 2       # k rows owned per core
INV_SQRT_D = 1.0 / math.sqrt(D)
BF16 = ml_dtypes.bfloat16


# ---------------------------------------------------------------------------
# Environment patches (compiler workarounds + profiling hook shim)
# ---------------------------------------------------------------------------

def _install_patches():
    import json as _json
    import concourse.bass as bass

    if not getattr(bass.Bass, "_mw_patched", False):
        _orig_to_json = bass.Bass.to_json_bytes

        def to_json_bytes(self):
            # This walrus build rejects any instruction carrying more than one
            # sync wait ("Too many sync wait commands").  Split extra waits
            # onto single-wait NoOps inserted just before the instruction (the
            # engine executes them in order, so semantics are unchanged).
            raw = _orig_to_json(self)
            m = _json.loads(raw)
            ctr = 0
            changed = False
            for fn in m.get("functions", []):
                for bb in fn.get("blocks", []):
                    out = []
                    for inst in bb.get("instructions", []):
                        si = inst.get("sync_info")
                        if si:
                            waits = si.get("on_wait") or []
                            if len(waits) > 1:
                                changed = True
                                for w in waits[:-1]:
                                    ctr += 1
                                    out.append({
                                        "debug": inst.get("debug", 0),
                                        "engine": inst["engine"],
                                        "ins": [],
                                        "outs": [],
                                        "name": f"I-mw{ctr}",
                                        "opcode": "NoOp",
                                        "text_hint": "mwsplit",
                                        "sync_info": {"on_wait": [w],
                                                      "on_update": []},
                                    })
                                si["on_wait"] = [waits[-1]]
                        out.append(inst)
                    bb["instructions"] = out
            if not changed:
                return raw
            return _json.dumps(m).encode()

        bass.Bass.to_json_bytes = to_json_bytes
        bass.Bass._mw_patched = True

    # Don't upload NEFF/trace artifacts anywhere; keep them local.
    import concourse.bass_utils as bu
    bu.upload_artifacts = lambda tmpdir: tmpdir


def _install_ntff_hook() -> bool:
    """Register the axon NTFF profiling hook (missing module in this image)."""
    try:
        import antenv.axon_hooks  # noqa: F401
        return True
    except ImportError:
        pass
    try:
        mod = types.ModuleType("antenv.axon_hooks")
        state = {"hook": None}
        mod.set_axon_ntff_profile_hook = lambda h: state.__setitem__("hook", h)
        mod.get_axon_ntff_profile_hook = lambda: state["hook"]
        sys.modules["antenv.axon_hooks"] = mod
        import antenv
        antenv.axon_hooks = mod
        from trn_agent_boot.trn_boot import _ntff_profile_via_ctypes
        mod.set_axon_ntff_profile_hook(
            _ntff_profile_via_ctypes("/opt/axon/libaxon_pjrt.so"))
        return True
    except Exception:
        return False


# ---------------------------------------------------------------------------
# Graph
# ---------------------------------------------------------------------------

def _build_graph():
    import concourse.bass as bass
    import concourse.bass_isa as bass_isa
    import concourse.mybir as mybir
    import concourse.tile as tile

    f32 = mybir.dt.float32
    bf16 = mybir.dt.bfloat16
    Exp = mybir.ActivationFunctionType.Exp

    nc = bass.Bass("TRN2", debug=False, num_devices=NCORES)

    embT_in = nc.dram_tensor("embT", [D, S], bf16, kind="ExternalInput")
    embkT_in = nc.dram_tensor("embkT", [D, KHALF], bf16, kind="ExternalInput")
    wkq_in = nc.dram_tensor("wkq", [D, 2 * D], bf16, kind="ExternalInput")
    wvT_in = nc.dram_tensor("wvT", [D, D], bf16, kind="ExternalInput")
    masks_in = nc.dram_tensor("masks", [128, NSLOT * CHUNK], bf16,
                              kind="ExternalInput")
    oav_d = nc.dram_tensor("oav", [S, D], f32, kind="ExternalOutput")
    ol_d = nc.dram_tensor("ol", [NSLOT, 128, CHUNK], f32,
                          kind="ExternalOutput")

    with tile.TileContext(nc) as tc:
        with (
            tc.tile_pool(name="wsb", bufs=1) as wsb,          # weights resident
            tc.tile_pool(name="eksb", bufs=1) as eksb,        # embk^T resident
            tc.tile_pool(name="vsb", bufs=1) as vsb,          # V resident
            tc.tile_pool(name="eqs", bufs=24) as eqs,         # embT q-side stream
            tc.tile_pool(name="mks", bufs=1) as mks,          # masks resident
            tc.tile_pool(name="wts", bufs=8) as wts,          # exp weights
            tc.tile_pool(name="outs", bufs=4) as outs,        # av out stage
            tc.tile_pool(name="lacc", bufs=2) as lacc,        # exp-sum accum
            tc.tile_pool(name="pmm", bufs=3, space="PSUM") as pmm,
        ):
            # resident weight tiles; DMA order = need order (M first).
            # wk|wq live side by side in one [128, 2048] tile per e-chunk:
            # half the DMA triggers (trigger issue paces the M phase).
            wkq_sb, wv_t = [], []
            for dc in range(8):
                t = wsb.tile([128, 2 * D], bf16, name=f"wkq{dc}",
                             tag=f"wkq{dc}")
                nc.sync.dma_start(t[:], wkq_in[dc * 128:(dc + 1) * 128, :])
                wkq_sb.append(t)
            embk_sb = []
            for dc in range(8):
                t = eksb.tile([128, KHALF], bf16, name=f"ek{dc}",
                              tag=f"ek{dc}")
                nc.sync.dma_start(t[:], embkT_in[dc * 128:(dc + 1) * 128, :])
                embk_sb.append(t)
            for dc in range(8):
                t = wsb.tile([128, D], bf16, name=f"wv{dc}", tag=f"wv{dc}")
                nc.sync.dma_start(t[:], wvT_in[dc * 128:(dc + 1) * 128, :])
                wv_t.append(t)

            # ---------------- MT = Wk^T @ Wq  [d', d] ----------------
            # scores = emb_q M emb_k^T with M[d,d'] = sum_e Wq[e,d] Wk[e,d'];
            # we materialize M^T (tiles [128d', 1024d]) as the lhsT source for
            # the K-side fold below.  The e-contraction loop is OUTER with 8
            # bank-aligned accumulators live at once, so the matmuls start as
            # soon as the first wk/wq e-chunk pair lands and stay paced with
            # the input DMA instead of waiting for the full 4MB.
            mt_sb = [wsb.tile([128, D], bf16, name=f"mt{ac}", tag=f"mt{ac}")
                     for ac in range(8)]
            with tc.tile_pool(name="pm4", bufs=2, space="PSUM") as pm4:
                for bb in range(2):
                    t3 = [pmm.tile([128, 1024], f32, name=f"pm{bb}_{i}",
                                   tag="mm") for i in range(3)]
                    q4 = [pm4.tile([128, 512], f32, name=f"pm4_{bb}_{i}",
                                   tag="m4") for i in range(2)]
                    accs = [t3[0][:, 0:512], t3[0][:, 512:1024],
                            t3[1][:, 0:512], t3[1][:, 512:1024],
                            t3[2][:, 0:512], t3[2][:, 512:1024],
                            q4[0][:], q4[1][:]]
                    for ec in range(8):
                        for ac in range(8):
                            nc.tensor.matmul(
                                accs[ac],
                                wkq_sb[ec][:, ac * 128:(ac + 1) * 128],
                                wkq_sb[ec][:, D + bb * 512:
                                            D + (bb + 1) * 512],
                                start=(ec == 0), stop=(ec == 7))
                    for ac in range(8):
                        nc.scalar.copy(mt_sb[ac][:, bb * 512:(bb + 1) * 512],
                                       accs[ac])

            # ---------------- V projection (my k rows), SBUF resident -------
            v_sb = []
            for sr in range(16):
                psum = pmm.tile([128, 1024], f32, name=f"pv{sr}", tag="mm")
                col = sr * 128
                for eb in range(2):
                    for dc in range(8):
                        nc.tensor.matmul(
                            psum[:, eb * 512:(eb + 1) * 512],
                            embk_sb[dc][:, col:col + 128],
                            wv_t[dc][:, eb * 512:(eb + 1) * 512],
                            start=(dc == 0), stop=(dc == 7))
                t = vsb.tile([128, 1024], bf16, name=f"v{sr}", tag=f"v{sr}")
                nc.scalar.copy(t[:], psum[:])
                v_sb.append(t)

            # ---------------- KP = M @ embk^T  [d, k] (scores lhsT) ---------
            # KP[d,k] = sum_d' MT[d',d] embk^T[d',k].  One [128d, 2048k]
            # tile per d-chunk, aliasing the dead wkq buffers; kh=0 (k tiles
            # 0..7) computed first.
            kp2 = [wsb.tile([128, 2 * D], bf16, name=f"kp{dc}",
                            tag=f"wkq{dc}") for dc in range(8)]
            for kh in range(2):
                for dc in range(8):
                    psum = pmm.tile([128, 1024], f32, name=f"pk{kh}_{dc}",
                                    tag="mm")
                    for kb in range(2):
                        koff = kh * 1024 + kb * 512
                        for ec in range(8):
                            nc.tensor.matmul(
                                psum[:, kb * 512:(kb + 1) * 512],
                                mt_sb[ec][:, dc * 128:(dc + 1) * 128],
                                embk_sb[ec][:, koff:koff + 512],
                                start=(ec == 0), stop=(ec == 7))
                    # kh=1 copies trail into the attention phase; keep them
                    # off the scalar engine so the first slots' exp calls
                    # aren't queued behind them.
                    dst = kp2[dc][:, kh * 1024:(kh + 1) * 1024]
                    if kh == 0:
                        nc.scalar.copy(dst, psum[:])
                    else:
                        nc.vector.tensor_copy(dst, psum[:])

            # ---------------- attention ----------------
            # slot j = query chunk j (rows 256j..256j+255); k tiles 0..j of
            # this core's half; tile kt lives in kp2[dc] at column block
            # kt*128 and v_sb[kt].  Only kt==j is diagonal-masked.
            #
            # Software-pipelined one k-tile deep AND one slot deep: each AV
            # group is emitted after the next scores+exp issue, so the tensor
            # engine never waits on the scalar exp — including at slot
            # boundaries, where the previous slot's last AV group (and its
            # output staging) runs after the next slot's first scores.
            # scores PSUM pool opens only now: during the M phase its banks
            # were lent to the 8-accumulator pm4 scope.  The softmax
            # denominators never touch PSUM (vector-summed in SBUF, folded on
            # the host), so the freed banks buy a third scores buffer.
            _ps_cm = tc.tile_pool(name="ps", bufs=2, space="PSUM")
            ps_pool = _ps_cm.__enter__()

            mk_all = mks.tile([128, NSLOT * CHUNK], bf16, name="mka",
                              tag="mka")
            nc.sync.dma_start(mk_all[:], masks_in[:, :])

            def emit_av(av, j, wt, kt):
                first, last = kt == 0, kt == j
                vt = v_sb[kt]
                for qs in range(2):
                    wslice = wt[:, qs * 128:(qs + 1) * 128]
                    for eb in range(2):
                        nc.tensor.matmul(
                            av[qs][:, eb * 512:(eb + 1) * 512], wslice,
                            vt[:, eb * 512:(eb + 1) * 512],
                            start=first, stop=last)

            def finish_slot(prev):
                j, w_acc, av, wt, kt = prev
                emit_av(av, j, wt, kt)
                # softmax denominator: exp-weights were elementwise-summed
                # over k-tiles on vector (f32).  Ship the [128, 256] partial
                # tile; the host folds the 128 partition lanes together with
                # the pair combine.
                nc.gpsimd.dma_start(ol_d[j, :, :], w_acc[:])
                for qs in range(2):
                    o_sb = outs.tile([128, 1024], f32, name=f"o{j}_{qs}",
                                     tag="outs")
                    nc.vector.tensor_copy(o_sb[:], av[qs][:])
                    row = (j * 2 + qs) * 128
                    nc.gpsimd.dma_start(oav_d[row:row + 128, :], o_sb[:])

            def scores_exp(eq, j, kt, w_acc):
                s_ps = ps_pool.tile([128, CHUNK], f32, name=f"s{j}_{kt}",
                                    tag="s")
                for dc in range(8):
                    nc.tensor.matmul(
                        s_ps[:], kp2[dc][:, kt * 128:(kt + 1) * 128],
                        eq[dc][:], start=(dc == 0), stop=(dc == 7))
                wt = wts.tile([128, CHUNK], bf16, name=f"w{j}_{kt}",
                              tag="wts")
                nc.scalar.activation(wt[:], s_ps[:], Exp, bias=0.0,
                                     scale=INV_SQRT_D)
                if kt == j:
                    nc.vector.tensor_mul(
                        wt[:], wt[:], mk_all[:, j * CHUNK:(j + 1) * CHUNK])
                if kt == 0:
                    nc.vector.tensor_copy(w_acc[:], wt[:])
                else:
                    nc.vector.tensor_add(w_acc[:], w_acc[:], wt[:])
                return wt

            # Dense slots interleave with thin ones: slot 7 first warms
            # the pipeline while the KP evacuation copies drain (it needs
            # only the kh=0 KP half), and each thin slot's boundary copies
            # drain behind a following dense slot's compute.
            order = [7, 0, 8, 1, 9, 2, 10, 3, 11, 4, 12, 5, 13, 6, 14, 15]
            prev = None
            for j in order:
                eq = []
                for dc in range(8):
                    t = eqs.tile([128, CHUNK], bf16, name=f"eq{j}_{dc}",
                                 tag="eqs")
                    nc.sync.dma_start(
                        t[:], embT_in[dc * 128:(dc + 1) * 128,
                                      j * CHUNK:(j + 1) * CHUNK])
                    eq.append(t)

                w_acc = lacc.tile([128, CHUNK], f32, name=f"wa{j}", tag="wacc")
                wt0 = scores_exp(eq, j, 0, w_acc)
                if prev is not None:
                    finish_slot(prev)

                av = [pmm.tile([128, 1024], f32, name=f"av{j}_{qs}", tag="mm")
                      for qs in range(2)]

                pend = (wt0, 0)
                for kt in range(1, j + 1):
                    wt = scores_exp(eq, j, kt, w_acc)
                    emit_av(av, j, *pend)
                    pend = (wt, kt)
                prev = (j, w_acc, av) + pend
            finish_slot(prev)

            _ps_cm.__exit__(None, None, None)

    return nc


_CACHED = {}


def _get_graph():
    if "nc" not in _CACHED:
        _install_patches()
        _CACHED["nc"] = _build_graph()
    return _CACHED["nc"]


# ---------------------------------------------------------------------------
# Host-side staging
# ---------------------------------------------------------------------------

def _masks(parity):
    m = np.zeros((NSLOT, 128, CHUNK), dtype=np.float32)
    for j in range(NSLOT):
        p = np.arange(128)[:, None]
        x = np.arange(CHUNK)[None, :]
        m[j] = ((j * CHUNK + x) >= ((2 * j + parity) * 128 + p))
    # device layout: [128 k-partitions, slot-major free dim]
    return np.ascontiguousarray(
        m.transpose(1, 0, 2).reshape(128, NSLOT * CHUNK)).astype(BF16)


def kernel(embeddings, Wq, Wk, Wv):
    embeddings = np.asarray(embeddings, dtype=np.float32)
    Wq = np.asarray(Wq, dtype=np.float32)
    Wk = np.asarray(Wk, dtype=np.float32)
    Wv = np.asarray(Wv, dtype=np.float32)

    nc = _get_graph()
    from concourse.bass_utils import run_bass_kernel_spmd

    wkq = np.ascontiguousarray(
        np.concatenate([Wk, Wq], axis=1)).astype(BF16)
    wvT = np.ascontiguousarray(Wv.T).astype(BF16)
    masks_by_par = [_masks(0), _masks(1)]

    in_maps = []
    for c in range(NCORES):
        b, par = divmod(c, 2)
        emb_b = embeddings[b]
        embT = np.ascontiguousarray(emb_b.T).astype(BF16)
        # my k rows: interleaved 128-row tiles (2t+par for t in 0..15)
        embk = np.concatenate(
            [emb_b[(2 * t + par) * 128:(2 * t + par) * 128 + 128]
             for t in range(16)], axis=0)
        embkT = np.ascontiguousarray(embk.T).astype(BF16)
        in_maps.append({
            "embT": embT,
            "embkT": embkT,
            "wkq": wkq,
            "wvT": wvT,
            "masks": masks_by_par[par],
        })

    trace = bool(int(os.environ.get("BASS_KERNEL_TRACE", "0")))
    kwargs = {}
    if trace:
        kwargs["trace"] = _install_ntff_hook()

    res = run_bass_kernel_spmd(nc, in_maps, core_ids=list(range(NCORES)),
                               **kwargs)
    _CACHED["last_result"] = res

    out = np.empty((B, S, D), dtype=np.float32)
    for b in range(B):
        r0, r1 = res.results[2 * b], res.results[2 * b + 1]
        av = (r0["oav"].astype(np.float32)
              + r1["oav"].astype(np.float32))          # [S, D]
        # fold the per-core k-partition lanes and the pair halves
        l = (r0["ol"] + r1["ol"]).sum(axis=1)           # [16, 256], q-major
        out[b] = av / l.reshape(S, 1)
    return out
